# revision 24
# baseline (speedup 1.0000x reference)
"""Causal MHA (B=2, S=2048, D=2048, H=16) on 8 TRN2 NeuronCores.

Sharding: heads split across cores (2 heads/core, both batches). Each core:
  1. qk^T GEMM  : qT/kT in [feat(=head dim) partitions, token free] layout
  2. V GEMM     : V in natural [token partitions, feat free] layout
  3. causal attention (scoresT layout; causal mask applied as an additive
     -60000 bias matmul inside the scores accumulation group; softmax
     denominator accumulated on the vector engine and reduced with one
     ones-matmul per block; normalization broadcast via gpsimd
     partition_broadcast; score pipeline runs 2 chunk-pairs ahead across
     block boundaries so the PE never idles)
  4. AllToAll   : heads -> token-slice redistribution of attention output
  5. out-proj   : full Wout (SBUF-resident) on this core's 512-token slice

Phase 4 of iteration i is emitted after the collective of iteration i+1 is
triggered (a2a buffers are parity double-buffered), so in the unrolled
steady state the collective overlaps compute and the PE stays busy.

All on-chip data is float16 (PSUM accumulation in f32), which runs the PE
at full rate, halves DMA/SBUF traffic vs f32, and keeps rel err ~1e-3.
Host passes x^T, per-core W shards (attn scale folded into Wq), masks,
ones/identity helpers. Output is assembled on host from the 8 transposed
f16 token slices.
"""
import time

import numpy as np

import concourse.bacc as bacc
import concourse.mybir as mybir
import concourse.tile as tile
from concourse import bass_utils

# ---- problem constants (hardcoded; must match the reference) ----
B, S, D_MODEL, H = 2, 2048, 2048, 16
HEAD_DIM = 128
N_CORES = 8
CORE_IDS = list(range(N_CORES))
T = B * S                      # 4096 flattened tokens
HPC = H // N_CORES             # 2 heads per core
TOKB = 512                     # token block for phase-1 GEMM streaming
NTB = T // TOKB                # 8
NKC = D_MODEL // 128           # 16 contraction chunks of d_model
SQB = 256                      # sq block width in attention
NJ = S // SQB                  # 8 sq blocks per batch
NSK = S // 128                 # 16 sk chunks per batch
TSL = T // N_CORES             # 512-token output slice per core

F16 = mybir.dt.float16
F32 = mybir.dt.float32
EXPF = mybir.ActivationFunctionType.Exp


def build(iters: int = 1, phases: str = "1234"):
    nc = bacc.Bacc("TRN2", target_bir_lowering=False, debug=False,
                   num_devices=N_CORES)

    xT_d = nc.dram_tensor("xT", [D_MODEL, T], F16, kind="ExternalInput").ap()
    wqk_d = nc.dram_tensor("wqk", [D_MODEL, 4 * 128], F16, kind="ExternalInput").ap()
    wv_d = nc.dram_tensor("wv", [D_MODEL, 2 * 128], F16, kind="ExternalInput").ap()
    wout_d = nc.dram_tensor("wout", [NKC, NKC, 128, 128], F16, kind="ExternalInput").ap()
    nmask_d = nc.dram_tensor("nmasks", [2, 128, SQB], F16, kind="ExternalInput").ap()
    ones_d = nc.dram_tensor("ones", [128, 128], F16, kind="ExternalInput").ap()
    ident_d = nc.dram_tensor("ident", [128, 128], F16, kind="ExternalInput").ap()
    outT_d = nc.dram_tensor("outT", [D_MODEL, TSL], F16, kind="ExternalOutput").ap()

    # parity double-buffered internal DRAM for the all-to-all
    a2a_in = nc.dram_tensor("a2a_in", [2, N_CORES, HPC * 128, TSL], F16).ap()
    a2a_out = nc.dram_tensor("a2a_out", [2, N_CORES, HPC * 128, TSL], F16).ap()

    with tile.TileContext(nc) as tc:
        with tc.tile_pool(name="persist", bufs=1) as pp:
            nmasks = pp.tile([128, 2, SQB], F16)
            ones = pp.tile([128, 128], F16)
            ident = pp.tile([128, 128], F16)
            wqk = pp.tile([128, NKC, 4 * 128], F16)
            wv = pp.tile([128, NKC, 2 * 128], F16)
            wout = pp.tile([128, NKC, NKC, 128], F16)  # [p, m, kc, n]
            nc.sync.dma_start(out=nmasks[:], in_=nmask_d.rearrange("r p q -> p r q"))
            nc.sync.dma_start(out=ones[:], in_=ones_d[:])
            nc.sync.dma_start(out=ident[:], in_=ident_d[:])
            nc.sync.dma_start(out=wqk[:], in_=wqk_d.rearrange("(k p) n -> p k n", p=128))
            nc.sync.dma_start(out=wv[:], in_=wv_d.rearrange("(k p) n -> p k n", p=128))
            for m in range(NKC):
                nc.gpsimd.dma_start(out=wout[:, m, :, :],
                                    in_=wout_d[m].rearrange("k p n -> p k n"))

            tensors = dict(xT_d=xT_d, wqk=wqk, wv=wv, wout=wout, outT_d=outT_d,
                           a2a_in=a2a_in, a2a_out=a2a_out, nmasks=nmasks,
                           ones=ones, ident=ident)
            prev_par = None
            for it in range(iters):
                par = it % 2
                of_prev = None
                if prev_par is not None and "4" in phases:
                    of_prev = _phase4_load(nc, pp, tensors, prev_par, phases)
                _body(nc, tc, pp, tensors, par, prev_par, phases)
                if of_prev is not None:
                    _phase4_compute(nc, tc, tensors, of_prev)
                prev_par = par
            if prev_par is not None and "4" in phases:
                of_last = _phase4_load(nc, pp, tensors, prev_par, phases)
                _phase4_compute(nc, tc, tensors, of_last)

    nc.compile()
    return nc


def _body(nc, tc, pp, tensors, par, prev_par, phases="1234"):
    noexp = "E" in phases
    noden = "D" in phases
    nomask = "M" in phases
    xT_d = tensors["xT_d"]
    wqk, wv = tensors["wqk"], tensors["wv"]
    a2a_in = tensors["a2a_in"]
    nmasks, ones, ident = tensors["nmasks"], tensors["ones"], tensors["ident"]

    with tc.tile_pool(name="qkv", bufs=1) as qkvp, \
         tc.tile_pool(name="psum", bufs=1, space="PSUM") as psp:
        # persistent activations for this iteration
        qkT = qkvp.tile([128, 4, T], F16)       # [d, (q0,q1,k0,k1), tok]
        v_sb = qkvp.tile([128, T // 128, 2 * 128], F16)  # [tok%128, chunk, feat]

        # ---------------- phase 1: QKV projection ----------------
        xT_r = xT_d.rearrange("(k p) t -> p k t", p=128)
        for tb in range(NTB):
            xt = qkvp.tile([128, NKC, TOKB], F16, tag="xt", bufs=2)
            nc.sync.dma_start(out=xt[:], in_=xT_r[:, :, tb * TOKB:(tb + 1) * TOKB])
            tok0 = tb * TOKB
            # q/k transposed GEMM, m-pairs: psum[feat, tok] += wqk.T @ xt
            for mp in range(2):
                ps = psp.tile([128, 2, TOKB], F32, tag="mm", bufs=2)
                for mi in range(2):
                    m = mp * 2 + mi
                    for kc in range(NKC):
                        nc.tensor.matmul(ps[:, mi, :],
                                         wqk[:, kc, m * 128:(m + 1) * 128],
                                         xt[:, kc, :],
                                         start=(kc == 0), stop=(kc == NKC - 1))
                with nc.allow_low_precision(reason="f16 activations"):
                    nc.vector.tensor_copy(qkT[:, mp * 2:mp * 2 + 2, tok0:tok0 + TOKB],
                                          ps[:])
            # V natural GEMM, ti-pairs: psum[tok, feat] += xt.T @ wv
            for tp in range(2):
                pv = psp.tile([128, 2, 2 * 128], F32, tag="acc", bufs=2)
                for ti in range(2):
                    t128 = (tp * 2 + ti) * 128
                    for kc in range(NKC):
                        nc.tensor.matmul(pv[:, ti, :],
                                         xt[:, kc, t128:t128 + 128],
                                         wv[:, kc, :],
                                         start=(kc == 0), stop=(kc == NKC - 1))
                with nc.allow_low_precision(reason="f16 activations"):
                    nc.vector.tensor_copy(v_sb[:, tb * 4 + tp * 2:tb * 4 + tp * 2 + 2, :],
                                          pv[:])

        # ---------------- phase 2: causal attention ----------------
        # SQB=256 query blocks; sk chunks in PAIRS: one psum tile
        # [128, 2(chunk), 2(head), 256] = 2 banks holds the 4 score matmuls
        # of a chunk pair, exp'd by a single ACT op. Scores are emitted 2
        # pairs ahead of the exp->AV consumers, across block boundaries, so
        # the PE never idles and stays at max p-state.
        if "2" not in phases:
            return
        with tc.tile_pool(name="p2", bufs=1) as p2:
            blocks = [(b, j) for j in range(NJ) for b in range(B)]
            flat = [(b, j, cp) for (b, j) in blocks for cp in range(j + 1)]
            LOOKAHEAD = 2
            sts = {}
            next_emit = [0]

            def emit_scores_upto(idx):
                while next_emit[0] <= min(idx, len(flat) - 1):
                    b, j, cp = flat[next_emit[0]]
                    st = psp.tile([128, 2, 2, SQB], F32, tag="mm", bufs=2,
                                  name="st")
                    for ci in range(2):
                        c = 2 * cp + ci
                        diag = c >= 2 * j and not nomask
                        for h in range(HPC):
                            nc.tensor.matmul(
                                st[:, ci, h, :],
                                qkT[:, 2 + h, b * S + c * 128: b * S + (c + 1) * 128],
                                qkT[:, h, b * S + j * SQB: b * S + (j + 1) * SQB],
                                start=True, stop=not diag)
                            if diag:
                                nc.tensor.matmul(st[:, ci, h, :], ident[:],
                                                 nmasks[:, c - 2 * j, :],
                                                 start=False, stop=True)
                    sts[next_emit[0]] = st
                    next_emit[0] += 1

            def emit_epilogue(b, j, o_acc, esum):
                g = b * (NJ // 2) + j // 2
                off = (j % 2) * SQB
                if noden:
                    o_sb = p2.tile([128, 2, SQB], F16, tag="osb", bufs=2,
                                   name="osb")
                    with nc.allow_low_precision(reason="timing variant only"):
                        nc.vector.tensor_copy(o_sb[:], o_acc[:, :, :SQB])
                else:
                    # den and bc live in o_acc's unused bank halves
                    # [*, h, SQB:2*SQB]: the o_acc accumulation groups are
                    # closed, so these are sequential groups in the same
                    # banks (legal), costing no extra psum
                    den = o_acc[0:1, :, SQB:2 * SQB]
                    for h in range(HPC):
                        nc.tensor.matmul(den[:, h, :], ones[:, 0:1],
                                         esum[:, h, :], start=True, stop=True)
                    rec = p2.tile([1, 2, SQB], F16, tag="rec", bufs=2, name="rec")
                    with nc.allow_low_precision(reason="softmax denom recip"):
                        nc.vector.reciprocal(rec[:], den[:])
                    bc_sb = p2.tile([128, 2, SQB], F16, tag="bcsb", bufs=2,
                                    name="bcsb")
                    nc.gpsimd.partition_broadcast(bc_sb[:], rec[:])
                    o_sb = p2.tile([128, 2, SQB], F16, tag="osb", bufs=2,
                                   name="osb")
                    with nc.allow_low_precision(reason="f16 attn output"):
                        nc.vector.tensor_mul(o_sb[:], o_acc[:, :, :SQB], bc_sb[:])
                for h in range(HPC):
                    nc.scalar.dma_start(
                        out=a2a_in[par, g, h * 128:(h + 1) * 128, off:off + SQB],
                        in_=o_sb[:, h, :])

            pending = None
            o_acc = esum = None
            for idx, (b, j, cp) in enumerate(flat):
                ncp = j + 1
                emit_scores_upto(idx + LOOKAHEAD)
                if cp == 0:
                    o_acc = psp.tile([128, 2, 2 * SQB], F32, tag="acc",
                                     bufs=2, name="oacc")
                    esum = None if noden else p2.tile(
                        [128, 2, SQB], F16, tag="esum", bufs=2, name="esum")
                    if pending is not None:
                        emit_epilogue(*pending)
                        pending = None
                st = sts.pop(idx)
                e = p2.tile([128, 2, 2, SQB], F16, tag="exp", bufs=6, name="e")
                if noexp:
                    with nc.allow_low_precision(reason="timing variant"):
                        nc.vector.tensor_copy(e[:], st[:])
                else:
                    nc.scalar.activation(e[:], st[:], EXPF)
                if not noden:
                    with nc.allow_low_precision(reason="f16 denom accum"):
                        if cp == 0:
                            nc.vector.tensor_add(esum[:], e[:, 0, :, :],
                                                 e[:, 1, :, :])
                        else:
                            nc.vector.tensor_add(esum[:], esum[:],
                                                 e[:, 0, :, :])
                            nc.vector.tensor_add(esum[:], esum[:],
                                                 e[:, 1, :, :])
                for ci in range(2):
                    c = 2 * cp + ci
                    for h in range(HPC):
                        nc.tensor.matmul(
                            o_acc[:, h, :SQB],
                            v_sb[:, b * NSK + c, h * 128:(h + 1) * 128],
                            e[:, ci, h, :], start=(c == 0),
                            stop=(c == 2 * ncp - 1))
                if cp == ncp - 1:
                    pending = (b, j, o_acc, esum)
            emit_epilogue(*pending)

    # ---------------- phase 3: all-to-all ----------------
    if "3" in phases:
        nc.gpsimd.collective_compute(
            "AllToAll", mybir.AluOpType.bypass, replica_groups=[CORE_IDS],
            ins=[tensors["a2a_in"][par]], outs=[tensors["a2a_out"][par]])


def _phase4_load(nc, pp, tensors, par, phases):
    """Emit the of loads early (descriptors wait on the collective sem)."""
    a2a_out = tensors["a2a_out"] if "3" in phases else tensors["a2a_in"]
    of = pp.tile([128, NKC, TSL], F16, tag="of", bufs=1, name="of")
    a2a_r = a2a_out[par].rearrange("g f t -> (g f) t").rearrange(
        "(k p) t -> p k t", p=128)
    for kc in range(NKC):
        nc.scalar.dma_start(out=of[:, kc, :], in_=a2a_r[:, kc, :])
    return of


def _phase4_compute(nc, tc, tensors, of):
    wout = tensors["wout"]
    outT_d = tensors["outT_d"]
    with tc.tile_pool(name="p4", bufs=1) as p4, \
         tc.tile_pool(name="psum4", bufs=1, space="PSUM") as psp4:
        for m in range(NKC):
            po = psp4.tile([128, TSL], F32, tag="po", bufs=4)
            for kc in range(NKC):
                nc.tensor.matmul(po[:], wout[:, m, kc, :], of[:, kc, :],
                                 start=(kc == 0), stop=(kc == NKC - 1))
            ot = p4.tile([128, TSL], F16, tag="ot", bufs=4)
            with nc.allow_low_precision(reason="f16 final output"):
                nc.scalar.copy(ot[:], po[:])
            nc.scalar.dma_start(out=outT_d[m * 128:(m + 1) * 128, :], in_=ot[:])


def _host_inputs(x, Wqkv, Wout):
    xT = np.ascontiguousarray(x.reshape(T, D_MODEL).T).astype(np.float16)
    scale = np.float32(HEAD_DIM ** -0.5)
    # additive causal mask: -60000 where query q < key row i + 128*r
    nmasks = np.zeros((2, 128, SQB), dtype=np.float16)
    for r in range(2):
        for i in range(128):
            lo = i + 128 * r
            if lo > 0:
                nmasks[r, i, :min(lo, SQB)] = -60000.0
    ones = np.ones((128, 128), dtype=np.float16)
    ident = np.eye(128, dtype=np.float16)
    # [m, k, 128, 128]: tile (k,m) of Wout, so each m-chunk load is contiguous
    Wout_t = np.ascontiguousarray(
        Wout.astype(np.float32).reshape(NKC, 128, NKC, 128).transpose(2, 0, 1, 3)
    ).astype(np.float16)

    in_maps = []
    for c in range(N_CORES):
        cols_q = [Wqkv[:, (2 * c + h) * 128:(2 * c + h + 1) * 128] for h in range(HPC)]
        cols_k = [Wqkv[:, D_MODEL + (2 * c + h) * 128:D_MODEL + (2 * c + h + 1) * 128]
                  for h in range(HPC)]
        cols_v = [Wqkv[:, 2 * D_MODEL + (2 * c + h) * 128:2 * D_MODEL + (2 * c + h + 1) * 128]
                  for h in range(HPC)]
        wqk = np.concatenate([c_ * scale for c_ in cols_q] + cols_k, axis=1)
        wv = np.concatenate(cols_v, axis=1)
        in_maps.append({
            "xT": xT,
            "wqk": np.ascontiguousarray(wqk).astype(np.float16),
            "wv": np.ascontiguousarray(wv).astype(np.float16),
            "wout": Wout_t,
            "nmasks": nmasks,
            "ones": ones,
            "ident": ident,
        })
    return in_maps


_NC_CACHE = {}


def _get_nc(iters=1, phases="1234"):
    key = (iters, phases)
    if key not in _NC_CACHE:
        _NC_CACHE[key] = build(iters, phases)
    return _NC_CACHE[key]


def kernel(x, Wqkv, Wout):
    x = np.asarray(x, dtype=np.float32)
    Wqkv = np.asarray(Wqkv, dtype=np.float32)
    Wout = np.asarray(Wout, dtype=np.float32)
    nc = _get_nc(1)
    in_maps = _host_inputs(x, Wqkv, Wout)
    res = None
    for attempt in range(3):
        try:
            res = bass_utils.run_bass_kernel_spmd(nc, in_maps, CORE_IDS)
            break
        except Exception:
            # transient NRT_EXEC_UNIT_UNRECOVERABLE after heavy prior device
            # activity recovers on retry; re-raise if persistent
            if attempt == 2:
                raise
            time.sleep(20)
    outT = np.concatenate([res.results[c]["outT"] for c in range(N_CORES)], axis=1)
    return np.ascontiguousarray(outT.T.astype(np.float32)).reshape(B, S, D_MODEL)


# revision 25
# speedup vs baseline: 1.0780x; 1.0780x over previous
"""Causal MHA (B=2, S=2048, D=2048, H=16) on 8 TRN2 NeuronCores.

Sharding: heads split across cores (2 heads/core, both batches). Each core:
  1. qk^T GEMM  : qT/kT in [feat(=head dim) partitions, token free] layout
  2. V GEMM     : V in natural [token partitions, feat free] layout
  3. causal attention (scoresT layout; causal mask applied as an additive
     -60000 bias matmul inside the scores accumulation group; softmax
     denominator accumulated on the vector engine and reduced with one
     ones-matmul per block; normalization broadcast via gpsimd
     partition_broadcast; score pipeline runs 2 chunk-pairs ahead across
     block boundaries so the PE never idles)
  4. AllToAll   : heads -> token-slice redistribution of attention output
  5. out-proj   : full Wout (SBUF-resident) on this core's 512-token slice

Phase 4 of iteration i is emitted after the collective of iteration i+1 is
triggered (a2a buffers are parity double-buffered), so in the unrolled
steady state the collective overlaps compute and the PE stays busy.

All on-chip data is float16 (PSUM accumulation in f32), which runs the PE
at full rate, halves DMA/SBUF traffic vs f32, and keeps rel err ~1e-3.
Host passes x^T, per-core W shards (attn scale folded into Wq), masks,
ones/identity helpers. Output is assembled on host from the 8 transposed
f16 token slices.
"""
import time

import numpy as np

import concourse.bacc as bacc
import concourse.mybir as mybir
import concourse.tile as tile
from concourse import bass_utils

# ---- problem constants (hardcoded; must match the reference) ----
B, S, D_MODEL, H = 2, 2048, 2048, 16
HEAD_DIM = 128
N_CORES = 8
CORE_IDS = list(range(N_CORES))
T = B * S                      # 4096 flattened tokens
HPC = H // N_CORES             # 2 heads per core
TOKB = 512                     # token block for phase-1 GEMM streaming
NTB = T // TOKB                # 8
NKC = D_MODEL // 128           # 16 contraction chunks of d_model
SQB = 256                      # sq block width in attention
NJ = S // SQB                  # 8 sq blocks per batch
NSK = S // 128                 # 16 sk chunks per batch
TSL = T // N_CORES             # 512-token output slice per core

F16 = mybir.dt.float16
F32 = mybir.dt.float32
EXPF = mybir.ActivationFunctionType.Exp


def build(iters: int = 1, phases: str = "1234"):
    nc = bacc.Bacc("TRN2", target_bir_lowering=False, debug=False,
                   num_devices=N_CORES)

    xT_d = nc.dram_tensor("xT", [D_MODEL, T], F16, kind="ExternalInput").ap()
    wqk_d = nc.dram_tensor("wqk", [D_MODEL, 4 * 128], F16, kind="ExternalInput").ap()
    wv_d = nc.dram_tensor("wv", [D_MODEL, 2 * 128], F16, kind="ExternalInput").ap()
    wout_d = nc.dram_tensor("wout", [NKC, NKC, 128, 128], F16, kind="ExternalInput").ap()
    nmask_d = nc.dram_tensor("nmasks", [2, 128, SQB], F16, kind="ExternalInput").ap()
    ones_d = nc.dram_tensor("ones", [128, 128], F16, kind="ExternalInput").ap()
    ident_d = nc.dram_tensor("ident", [128, 128], F16, kind="ExternalInput").ap()
    outT_d = nc.dram_tensor("outT", [D_MODEL, TSL], F16, kind="ExternalOutput").ap()

    # parity double-buffered internal DRAM for the all-to-all
    a2a_in = nc.dram_tensor("a2a_in", [2, N_CORES, HPC * 128, TSL], F16).ap()
    a2a_out = nc.dram_tensor("a2a_out", [2, N_CORES, HPC * 128, TSL], F16).ap()

    with tile.TileContext(nc) as tc:
        with tc.tile_pool(name="persist", bufs=1) as pp:
            nmasks = pp.tile([128, 2, SQB], F16)
            ones = pp.tile([128, 128], F16)
            ident = pp.tile([128, 128], F16)
            wqk = pp.tile([128, NKC, 4 * 128], F16)
            wv = pp.tile([128, NKC, 2 * 128], F16)
            wout = pp.tile([128, NKC, NKC, 128], F16)  # [p, m, kc, n]
            nc.sync.dma_start(out=nmasks[:], in_=nmask_d.rearrange("r p q -> p r q"))
            nc.sync.dma_start(out=ones[:], in_=ones_d[:])
            nc.sync.dma_start(out=ident[:], in_=ident_d[:])
            nc.sync.dma_start(out=wqk[:], in_=wqk_d.rearrange("(k p) n -> p k n", p=128))
            nc.sync.dma_start(out=wv[:], in_=wv_d.rearrange("(k p) n -> p k n", p=128))
            for m in range(NKC):
                nc.gpsimd.dma_start(out=wout[:, m, :, :],
                                    in_=wout_d[m].rearrange("k p n -> p k n"))

            tensors = dict(xT_d=xT_d, wqk=wqk, wv=wv, wout=wout, outT_d=outT_d,
                           a2a_in=a2a_in, a2a_out=a2a_out, nmasks=nmasks,
                           ones=ones, ident=ident)
            prev_par = None
            for it in range(iters):
                par = it % 2
                of_prev = None
                if prev_par is not None and "4" in phases:
                    of_prev = _phase4_load(nc, pp, tensors, prev_par, phases)
                _body(nc, tc, pp, tensors, par, prev_par, phases)
                if of_prev is not None:
                    _phase4_compute(nc, tc, tensors, of_prev)
                prev_par = par
            if prev_par is not None and "4" in phases:
                of_last = _phase4_load(nc, pp, tensors, prev_par, phases)
                _phase4_compute(nc, tc, tensors, of_last)

    nc.compile()
    return nc


def _body(nc, tc, pp, tensors, par, prev_par, phases="1234"):
    noexp = "E" in phases
    noden = "D" in phases
    nomask = "M" in phases
    xT_d = tensors["xT_d"]
    wqk, wv = tensors["wqk"], tensors["wv"]
    a2a_in = tensors["a2a_in"]
    nmasks, ones, ident = tensors["nmasks"], tensors["ones"], tensors["ident"]

    with tc.tile_pool(name="qkv", bufs=1) as qkvp, \
         tc.tile_pool(name="psum", bufs=1, space="PSUM") as psp:
        # persistent activations for this iteration
        qkT = qkvp.tile([128, 4, T], F16)       # [d, (q0,q1,k0,k1), tok]
        v_sb = qkvp.tile([128, T // 128, 2 * 128], F16)  # [tok%128, chunk, feat]

        # ---------------- phase 1: QKV projection ----------------
        xT_r = xT_d.rearrange("(k p) t -> p k t", p=128)
        for tb in range(NTB):
            xt = qkvp.tile([128, NKC, TOKB], F16, tag="xt", bufs=2)
            nc.sync.dma_start(out=xt[:], in_=xT_r[:, :, tb * TOKB:(tb + 1) * TOKB])
            tok0 = tb * TOKB
            # q/k transposed GEMM, m-pairs: psum[feat, tok] += wqk.T @ xt
            for mp in range(2):
                ps = psp.tile([128, 2, TOKB], F32, tag="mm", bufs=2)
                for mi in range(2):
                    m = mp * 2 + mi
                    for kc in range(NKC):
                        nc.tensor.matmul(ps[:, mi, :],
                                         wqk[:, kc, m * 128:(m + 1) * 128],
                                         xt[:, kc, :],
                                         start=(kc == 0), stop=(kc == NKC - 1))
                with nc.allow_low_precision(reason="f16 activations"):
                    nc.vector.tensor_copy(qkT[:, mp * 2:mp * 2 + 2, tok0:tok0 + TOKB],
                                          ps[:])
            # V natural GEMM, ti-pairs: psum[tok, feat] += xt.T @ wv
            for tp in range(2):
                pv = psp.tile([128, 2, 2 * 128], F32, tag="acc", bufs=2)
                for ti in range(2):
                    t128 = (tp * 2 + ti) * 128
                    for kc in range(NKC):
                        nc.tensor.matmul(pv[:, ti, :],
                                         xt[:, kc, t128:t128 + 128],
                                         wv[:, kc, :],
                                         start=(kc == 0), stop=(kc == NKC - 1))
                with nc.allow_low_precision(reason="f16 activations"):
                    nc.vector.tensor_copy(v_sb[:, tb * 4 + tp * 2:tb * 4 + tp * 2 + 2, :],
                                          pv[:])

        # ---------------- phase 2: causal attention ----------------
        # SQB=256 query blocks; sk chunks in PAIRS: one psum tile
        # [128, 2(chunk), 2(head), 256] = 2 banks holds the 4 score matmuls
        # of a chunk pair, exp'd by a single ACT op. Scores are emitted 2
        # pairs ahead of the exp->AV consumers, across block boundaries, so
        # the PE never idles and stays at max p-state.
        if "2" not in phases:
            return
        with tc.tile_pool(name="p2", bufs=1) as p2:
            blocks = [(b, j) for j in range(NJ) for b in range(B)]
            flat = [(b, j, cp) for (b, j) in blocks for cp in range(j + 1)]
            LOOKAHEAD = 2
            sts = {}
            next_emit = [0]

            def emit_scores_upto(idx):
                while next_emit[0] <= min(idx, len(flat) - 1):
                    b, j, cp = flat[next_emit[0]]
                    st = psp.tile([128, 2, 2, SQB], F32, tag="mm", bufs=2,
                                  name="st")
                    for ci in range(2):
                        c = 2 * cp + ci
                        diag = c >= 2 * j and not nomask
                        for h in range(HPC):
                            nc.tensor.matmul(
                                st[:, ci, h, :],
                                qkT[:, 2 + h, b * S + c * 128: b * S + (c + 1) * 128],
                                qkT[:, h, b * S + j * SQB: b * S + (j + 1) * SQB],
                                start=True, stop=not diag)
                            if diag:
                                nc.tensor.matmul(st[:, ci, h, :], ident[:],
                                                 nmasks[:, c - 2 * j, :],
                                                 start=False, stop=True)
                    sts[next_emit[0]] = st
                    next_emit[0] += 1

            def emit_epilogue(b, j, o_acc, esum):
                g = b * (NJ // 2) + j // 2
                off = (j % 2) * SQB
                if noden:
                    o_sb = p2.tile([128, 2, SQB], F16, tag="osb", bufs=2,
                                   name="osb")
                    with nc.allow_low_precision(reason="timing variant only"):
                        nc.vector.tensor_copy(o_sb[:], o_acc[:, :, :SQB])
                else:
                    # den and bc live in o_acc's unused bank halves
                    # [*, h, SQB:2*SQB]: the o_acc accumulation groups are
                    # closed, so these are sequential groups in the same
                    # banks (legal), costing no extra psum
                    den = o_acc[0:1, :, SQB:2 * SQB]
                    for h in range(HPC):
                        nc.tensor.matmul(den[:, h, :], ones[:, 0:1],
                                         esum[:, h, :], start=True, stop=True)
                    rec = p2.tile([1, 2, SQB], F16, tag="rec", bufs=2, name="rec")
                    with nc.allow_low_precision(reason="softmax denom recip"):
                        nc.vector.reciprocal(rec[:], den[:])
                    bc_sb = p2.tile([128, 2, SQB], F16, tag="bcsb", bufs=2,
                                    name="bcsb")
                    nc.gpsimd.partition_broadcast(bc_sb[:], rec[:])
                    o_sb = p2.tile([128, 2, SQB], F16, tag="osb", bufs=2,
                                   name="osb")
                    with nc.allow_low_precision(reason="f16 attn output"):
                        nc.vector.tensor_mul(o_sb[:], o_acc[:, :, :SQB], bc_sb[:])
                for h in range(HPC):
                    nc.sync.dma_start(
                        out=a2a_in[par, g, h * 128:(h + 1) * 128, off:off + SQB],
                        in_=o_sb[:, h, :])

            pending = None
            o_acc = esum = None
            for idx, (b, j, cp) in enumerate(flat):
                ncp = j + 1
                emit_scores_upto(idx + LOOKAHEAD)
                if cp == 0:
                    o_acc = psp.tile([128, 2, 2 * SQB], F32, tag="acc",
                                     bufs=2, name="oacc")
                    esum = None if noden else p2.tile(
                        [128, 2, SQB], F16, tag="esum", bufs=2, name="esum")
                    if pending is not None:
                        emit_epilogue(*pending)
                        pending = None
                st = sts.pop(idx)
                e = p2.tile([128, 2, 2, SQB], F16, tag="exp", bufs=7, name="e")
                if noexp:
                    with nc.allow_low_precision(reason="timing variant"):
                        nc.vector.tensor_copy(e[:], st[:])
                else:
                    nc.scalar.activation(e[:], st[:], EXPF)
                if not noden:
                    with nc.allow_low_precision(reason="f16 denom accum"):
                        if cp == 0:
                            nc.vector.tensor_add(esum[:], e[:, 0, :, :],
                                                 e[:, 1, :, :])
                        else:
                            nc.vector.tensor_add(esum[:], esum[:],
                                                 e[:, 0, :, :])
                            nc.vector.tensor_add(esum[:], esum[:],
                                                 e[:, 1, :, :])
                for ci in range(2):
                    c = 2 * cp + ci
                    for h in range(HPC):
                        nc.tensor.matmul(
                            o_acc[:, h, :SQB],
                            v_sb[:, b * NSK + c, h * 128:(h + 1) * 128],
                            e[:, ci, h, :], start=(c == 0),
                            stop=(c == 2 * ncp - 1))
                if cp == ncp - 1:
                    pending = (b, j, o_acc, esum)
            emit_epilogue(*pending)

    # ---------------- phase 3: all-to-all ----------------
    if "3" in phases:
        nc.gpsimd.collective_compute(
            "AllToAll", mybir.AluOpType.bypass, replica_groups=[CORE_IDS],
            ins=[tensors["a2a_in"][par]], outs=[tensors["a2a_out"][par]])


def _phase4_load(nc, pp, tensors, par, phases):
    """Emit the of loads early (descriptors wait on the collective sem)."""
    a2a_out = tensors["a2a_out"] if "3" in phases else tensors["a2a_in"]
    of = pp.tile([128, NKC, TSL], F16, tag="of", bufs=1, name="of")
    a2a_r = a2a_out[par].rearrange("g f t -> (g f) t").rearrange(
        "(k p) t -> p k t", p=128)
    for kc in range(NKC):
        nc.scalar.dma_start(out=of[:, kc, :], in_=a2a_r[:, kc, :])
    return of


def _phase4_compute(nc, tc, tensors, of):
    wout = tensors["wout"]
    outT_d = tensors["outT_d"]
    with tc.tile_pool(name="p4", bufs=1) as p4, \
         tc.tile_pool(name="psum4", bufs=1, space="PSUM") as psp4:
        for m in range(NKC):
            po = psp4.tile([128, TSL], F32, tag="po", bufs=4)
            for kc in range(NKC):
                nc.tensor.matmul(po[:], wout[:, m, kc, :], of[:, kc, :],
                                 start=(kc == 0), stop=(kc == NKC - 1))
            ot = p4.tile([128, TSL], F16, tag="ot", bufs=4)
            with nc.allow_low_precision(reason="f16 final output"):
                nc.scalar.copy(ot[:], po[:])
            nc.gpsimd.dma_start(out=outT_d[m * 128:(m + 1) * 128, :], in_=ot[:])


def _host_inputs(x, Wqkv, Wout):
    xT = np.ascontiguousarray(x.reshape(T, D_MODEL).T).astype(np.float16)
    scale = np.float32(HEAD_DIM ** -0.5)
    # additive causal mask: -60000 where query q < key row i + 128*r
    nmasks = np.zeros((2, 128, SQB), dtype=np.float16)
    for r in range(2):
        for i in range(128):
            lo = i + 128 * r
            if lo > 0:
                nmasks[r, i, :min(lo, SQB)] = -60000.0
    ones = np.ones((128, 128), dtype=np.float16)
    ident = np.eye(128, dtype=np.float16)
    # [m, k, 128, 128]: tile (k,m) of Wout, so each m-chunk load is contiguous
    Wout_t = np.ascontiguousarray(
        Wout.astype(np.float32).reshape(NKC, 128, NKC, 128).transpose(2, 0, 1, 3)
    ).astype(np.float16)

    in_maps = []
    for c in range(N_CORES):
        cols_q = [Wqkv[:, (2 * c + h) * 128:(2 * c + h + 1) * 128] for h in range(HPC)]
        cols_k = [Wqkv[:, D_MODEL + (2 * c + h) * 128:D_MODEL + (2 * c + h + 1) * 128]
                  for h in range(HPC)]
        cols_v = [Wqkv[:, 2 * D_MODEL + (2 * c + h) * 128:2 * D_MODEL + (2 * c + h + 1) * 128]
                  for h in range(HPC)]
        wqk = np.concatenate([c_ * scale for c_ in cols_q] + cols_k, axis=1)
        wv = np.concatenate(cols_v, axis=1)
        in_maps.append({
            "xT": xT,
            "wqk": np.ascontiguousarray(wqk).astype(np.float16),
            "wv": np.ascontiguousarray(wv).astype(np.float16),
            "wout": Wout_t,
            "nmasks": nmasks,
            "ones": ones,
            "ident": ident,
        })
    return in_maps


_NC_CACHE = {}


def _get_nc(iters=1, phases="1234"):
    key = (iters, phases)
    if key not in _NC_CACHE:
        _NC_CACHE[key] = build(iters, phases)
    return _NC_CACHE[key]


def kernel(x, Wqkv, Wout):
    x = np.asarray(x, dtype=np.float32)
    Wqkv = np.asarray(Wqkv, dtype=np.float32)
    Wout = np.asarray(Wout, dtype=np.float32)
    nc = _get_nc(1)
    in_maps = _host_inputs(x, Wqkv, Wout)
    res = None
    for attempt in range(3):
        try:
            res = bass_utils.run_bass_kernel_spmd(nc, in_maps, CORE_IDS)
            break
        except Exception:
            # transient NRT_EXEC_UNIT_UNRECOVERABLE after heavy prior device
            # activity recovers on retry; re-raise if persistent
            if attempt == 2:
                raise
            time.sleep(20)
    outT = np.concatenate([res.results[c]["outT"] for c in range(N_CORES)], axis=1)
    return np.ascontiguousarray(outT.T.astype(np.float32)).reshape(B, S, D_MODEL)
